# revision 25
# baseline (speedup 1.0000x reference)
"""Fused sparse-attention kernel for TRN2, SPMD over 8 NeuronCores.

Sharding: data-parallel over batch (32 -> 4 per core). Per core, the full
block (LayerNorm -> fused qkv -> per-head attention with gathered relative
position bias -> proj) is computed on-chip; attention probabilities never
touch HBM.

Softmax: softmax(S + B) is computed as exp(S + B - c); the gathered bias
B.T (precomputed on host from the tiny attn_biases table and the fixed
index map) is accumulated into the score PSUM by an identity-weight matmul
so the exp comes straight from PSUM with no extra element-wise pass;
row-sums are folded into the PV matmul via a ones-column appended to V.
The constant offset c gives fp16 headroom and cancels in the
normalization.

v2 schedule (vs baseline): the per-mc chain S -> exp -> mult -> PV is
software-pipelined at head (hp) granularity so the ACT engine (exp, the
bottleneck) runs back-to-back and the PE never sits behind a stalled PV:
S(mc+1) is emitted BEFORE PV(mc) on the tensor queue. Score PSUM tiles are
per-hp so exp(mc,hp0) frees banks while exp(mc,hp1) still runs. The softmax
reciprocals are batched ([2,N] per b, reciprocal_approx_fast) instead of 64
slow [1,N] reciprocal() calls, and PSUM drains moved off the ACT engine.
"""

import os
import sys

import numpy as np

for _p in ("/opt/trn_rl_repo", "/root/.axon_site/_ro/trn_rl_repo"):
    if os.path.isdir(_p) and _p not in sys.path:
        sys.path.insert(0, _p)

import concourse.bacc as bacc
import concourse.tile as tile
from concourse import bass_utils, mybir
from concourse.masks import make_identity

F32 = mybir.dt.float32
F16 = mybir.dt.float16

NCORES = 8
B_TOTAL = 32
NB = B_TOTAL // NCORES  # local batch per core
N = 1024
NT = 8        # 128-row tiles over n
DIM = 256
CC = 2        # 128-row chunks over DIM
H = 8
KD = 16
D = 64
MC = 8        # 128-row chunks over m
EPS = 1e-5
OFF = float(4.0 * np.log(2.0))  # exp offset for fp16 headroom (cancels)

MULT = mybir.AluOpType.mult
ADD = mybir.AluOpType.add


def _emit(tc, aps):
    nc = tc.nc
    x, wqk, wv, wp, bqk, bv, bp, etab, out = aps

    with tc.tile_pool(name="persist", bufs=1) as persist:
        # --- constants / weights resident in SBUF ---
        wqk_sb = persist.tile([128, CC, 4, 128], F16)
        nc.sync.dma_start(out=wqk_sb, in_=wqk.rearrange("cc ci jt j -> ci cc jt j"))
        wv_sb = persist.tile([128, CC, 512], F16)
        nc.sync.dma_start(out=wv_sb, in_=wv.rearrange("cc ci v -> ci cc v"))
        wp_sb = persist.tile([128, 4, 256], F16)
        nc.sync.dma_start(out=wp_sb, in_=wp.rearrange("cc ci c -> ci cc c"))
        bqk_sb = persist.tile([128, 4], F32)
        nc.sync.dma_start(out=bqk_sb, in_=bqk.rearrange("jt j -> j jt"))
        bv_sb = persist.tile([128, 512], F32)
        nc.sync.dma_start(out=bv_sb, in_=bv.partition_broadcast(128))
        bp_sb = persist.tile([128, 256], F32)
        nc.sync.dma_start(out=bp_sb, in_=bp.partition_broadcast(128))
        ident = persist.tile([128, 128], F16)
        make_identity(nc, ident)
        negoff = persist.tile([128, 1], F32)
        nc.vector.memset(negoff, -OFF)
        epsv = persist.tile([128, 1], F32)
        nc.vector.memset(epsv, EPS)

        qkT_l = []  # per-b [128, 4, 1024] f16: jt tiles (kT g0, qT g0, kT g1, qT g1)
        v_l = []    # per-b [128, NT, H, 65] f16: V rows + ones column per head
        ot_l = []   # per-b [128, 4, 1024] f16: O.T (dh on partitions, 4 chunks)

        # ---------------- phase 1: LN, xn.T, qkv projections ----------------
        with (
            tc.tile_pool(name="p1", bufs=2) as p1,
            tc.tile_pool(name="p1ps", bufs=2, space="PSUM") as p1ps,
        ):
            for b in range(NB):
                x_sb = p1.tile([128, NT, DIM], F32, tag="x", bufs=2)
                nc.sync.dma_start(
                    out=x_sb, in_=x[b].rearrange("(t p) c -> p t c", p=128)
                )
                xn_sb = p1.tile([128, NT, DIM], F16, tag="xn", bufs=2)
                for t in range(NT):
                    stats = p1.tile([128, 6], F32, tag="stats", bufs=3)
                    nc.vector.bn_stats(out=stats, in_=x_sb[:, t])
                    mv = p1.tile([128, 2], F32, tag="mv", bufs=3)
                    nc.vector.bn_aggr(out=mv, in_=stats)
                    rstd = p1.tile([128, 1], F32, tag="rstd", bufs=3)
                    nc.scalar.activation(
                        out=rstd, in_=mv[:, 1:2],
                        func=mybir.ActivationFunctionType.Sqrt,
                        bias=epsv, scale=1.0,
                    )
                    nc.vector.reciprocal(out=rstd, in_=rstd)
                    nc.vector.tensor_scalar(
                        out=xn_sb[:, t], in0=x_sb[:, t],
                        scalar1=mv[:, 0:1], scalar2=rstd,
                        op0=mybir.AluOpType.subtract, op1=mybir.AluOpType.mult,
                    )
                # xn.T via PE transpose; drain on ACT (phase 1 is DVE-bound,
                # ACT idles here)
                xnT = p1.tile([128, CC, N], F16, tag="xnt", bufs=2)
                for cc in range(CC):
                    for t in range(NT):
                        tp = p1ps.tile([128, 128], F16, tag="tp", bufs=2)
                        nc.tensor.transpose(
                            tp, xn_sb[:, t, cc * 128:(cc + 1) * 128], ident
                        )
                        nc.scalar.copy(
                            out=xnT[:, cc, t * 128:(t + 1) * 128], in_=tp
                        )
                # q.T / k.T, packed by 32-row strips per head (zeros padding)
                qkT = persist.tile([128, 4, N], F16, tag="qkT", bufs=NB, name="qkT")
                for jt in range(4):
                    qkp = p1ps.tile([128, N], F32, tag="qkp", bufs=2)
                    for nh in range(2):
                        for cc in range(CC):
                            nc.tensor.matmul(
                                qkp[:, nh * 512:(nh + 1) * 512],
                                lhsT=wqk_sb[:, cc, jt],
                                rhs=xnT[:, cc, nh * 512:(nh + 1) * 512],
                                start=(cc == 0), stop=(cc == CC - 1),
                            )
                    nc.scalar.activation(
                        out=qkT[:, jt], in_=qkp,
                        func=mybir.ActivationFunctionType.Identity,
                        bias=bqk_sb[:, jt:jt + 1], scale=1.0,
                    )
                qkT_l.append(qkT)
                # V (natural layout) + ones column, interleaved per head
                v_sb = persist.tile([128, NT, H, 65], F16, tag="v", bufs=NB,
                                    name="v_sb")
                nc.vector.memset(v_sb[:, :, :, 64:65], 1.0)
                for t in range(NT):
                    vp = p1ps.tile([128, 512], F32, tag="vp", bufs=2)
                    for cc in range(CC):
                        nc.tensor.matmul(
                            vp,
                            lhsT=xnT[:, cc, t * 128:(t + 1) * 128],
                            rhs=wv_sb[:, cc],
                            start=(cc == 0), stop=(cc == CC - 1),
                        )
                    nc.vector.tensor_tensor(
                        out=v_sb[:, t, :, 0:64],
                        in0=vp.rearrange("p (h d) -> p h d", d=64),
                        in1=bv_sb.rearrange("p (h d) -> p h d", d=64),
                        op=ADD,
                    )
                v_l.append(v_sb)

        # ---------------- phase 2: attention per head pair ----------------
        for b in range(NB):
            ot_l.append(persist.tile([128, 4, N], F16, tag="ot", bufs=NB,
                                     name="ot"))

        with (
            tc.tile_pool(name="p2", bufs=2) as p2,
            tc.tile_pool(name="p2ps", bufs=2, space="PSUM") as p2ps,
        ):
            for g in range(4):  # head pair {2g, 2g+1}
                # additive bias tiles B.T for this pair, quarter-granular so
                # the next g's DMA overlaps this g's tail
                e_tiles = {}
                for hp in range(2):
                    for q in range(2):
                        et = p2.tile([128, 4, N], F16, tag="e", bufs=4,
                                     name="et")
                        nc.sync.dma_start(
                            out=et,
                            in_=etab[2 * g + hp, 4 * q:4 * q + 4].rearrange(
                                "mc p n -> p mc n"),
                        )
                        e_tiles[(hp, q)] = et

                for b in range(NB):
                    # O'.T accumulators, one per head of the pair:
                    # [65, n] = V'.T @ P.T; row 64 carries the softmax sums
                    o_ts = [
                        p2ps.tile([65, N], F32, tag="o", bufs=2, name="o_ts")
                        for _ in range(2)
                    ]
                    s_tiles = {}

                    def emit_s(mc, b=b, s_tiles=s_tiles):
                        # both heads' score matmuls first (disjoint 32-row
                        # strips -> the PE row-tiles them concurrently),
                        # then the gathered bias B.T accumulated on top via
                        # identity-weight matmuls: exp reads S+B straight
                        # from PSUM, no DVE pass needed
                        ss = []
                        for hp in range(2):
                            h = 2 * g + hp
                            jt = 2 * (h // 4)
                            strip = 32 * (h % 4)
                            s = p2ps.tile([128, N], F32, tag="s", bufs=2,
                                          name="s_ps")
                            for nh in range(2):
                                nc.tensor.matmul(
                                    s[:, nh * 512:(nh + 1) * 512],
                                    lhsT=qkT_l[b][strip:strip + KD, jt,
                                                  mc * 128:(mc + 1) * 128],
                                    rhs=qkT_l[b][strip:strip + KD, jt + 1,
                                                 nh * 512:(nh + 1) * 512],
                                    start=True, stop=False,
                                    skip_group_check=True,
                                    tile_position=(strip, 0),
                                )
                            ss.append(s)
                        for hp in range(2):
                            for nh in range(2):
                                nc.tensor.matmul(
                                    ss[hp][:, nh * 512:(nh + 1) * 512],
                                    lhsT=ident,
                                    rhs=e_tiles[(hp, mc // 4)][
                                        :, mc % 4, nh * 512:(nh + 1) * 512],
                                    start=False, stop=True,
                                    skip_group_check=True,
                                )
                            s_tiles[(mc, hp)] = ss[hp]

                    emit_s(0)
                    for mc in range(MC):
                        # keep the PE queue fed: next mc's scores go ahead of
                        # this mc's PV (which waits on exp)
                        if mc + 1 < MC:
                            emit_s(mc + 1)
                        ps_hp = []
                        for hp in range(2):
                            ps = p2.tile([128, N], F16, tag="ps", bufs=4,
                                         name="ps")
                            nc.scalar.activation(
                                out=ps, in_=s_tiles.pop((mc, hp)),
                                func=mybir.ActivationFunctionType.Exp,
                                bias=negoff, scale=1.0,
                            )
                            ps_hp.append(ps)
                        for hp in range(2):
                            for nh in range(2):
                                nc.tensor.matmul(
                                    o_ts[hp][:, nh * 512:(nh + 1) * 512],
                                    lhsT=v_l[b][:, mc, 2 * g + hp],
                                    rhs=ps_hp[hp][:, nh * 512:(nh + 1) * 512],
                                    start=(mc == 0), stop=(mc == MC - 1),
                                    skip_group_check=True,
                                )
                    # Drain PSUM fast on DVE, then batched-normalize:
                    # one reciprocal_approx_fast([2,N]) replaces two slow
                    # f16 reciprocal([1,N]) calls.
                    # softmax sums land on partitions 0 and 32 of a shared
                    # collector (compute APs need 32-aligned partition bases)
                    # so ONE reciprocal_approx_fast covers both heads; rows
                    # 1..31 hold garbage and are never read. The broadcast
                    # source must sit at partition base 0, so each head's
                    # reciprocal row is copied into its own [1, N] tile.
                    raws = []
                    c32 = p2.tile([33, N], F32, tag="c32", bufs=2)
                    for hp in range(2):
                        raw = p2.tile([64, N], F16, tag="raw", bufs=4)
                        nc.vector.tensor_copy(out=raw, in_=o_ts[hp][0:64])
                        nc.vector.tensor_copy(
                            out=c32[32 * hp:32 * hp + 1], in_=o_ts[hp][64:65]
                        )
                        raws.append(raw)
                    r32 = p2.tile([33, N], F32, tag="r32", bufs=2)
                    nc.vector.reciprocal_approx_fast(out=r32, in_=c32)
                    for hp in range(2):
                        r1 = p2.tile([1, N], F16, tag="r1", bufs=4)
                        nc.vector.tensor_copy(
                            out=r1, in_=r32[32 * hp:32 * hp + 1])
                        rb = p2.tile([64, N], F16, tag="rb", bufs=4)
                        nc.gpsimd.partition_broadcast(rb, r1)
                        nc.vector.tensor_tensor(
                            out=ot_l[b][64 * hp:64 * hp + 64, g, :],
                            in0=raws[hp],
                            in1=rb,
                            op=MULT,
                        )

        # ---------------- phase 3: output projection ----------------
        with (
            tc.tile_pool(name="p3", bufs=2) as p3,
            tc.tile_pool(name="p3ps", bufs=4, space="PSUM") as p3ps,
        ):
            for b in range(NB):
                o_sb = p3.tile([128, NT, 256], F32, tag="osb", bufs=2)
                for nt in range(NT):
                    y = p3ps.tile([128, 256], F32, tag="y", bufs=4)
                    for cc2 in range(4):
                        nc.tensor.matmul(
                            y,
                            lhsT=ot_l[b][:, cc2, nt * 128:(nt + 1) * 128],
                            rhs=wp_sb[:, cc2],
                            start=(cc2 == 0), stop=(cc2 == 3),
                        )
                    nc.vector.tensor_tensor(
                        out=o_sb[:, nt], in0=y, in1=bp_sb, op=ADD
                    )
                nc.sync.dma_start(
                    out=out[b].rearrange("(t p) c -> p t c", p=128), in_=o_sb
                )


def build_module():
    nc = bacc.Bacc(
        "TRN2",
        target_bir_lowering=False,
        debug=False,
        enable_asserts=False,
        num_devices=NCORES,
    )
    x_t = nc.dram_tensor("x", [NB, N, DIM], F32, kind="ExternalInput")
    wqk_t = nc.dram_tensor("wqk", [CC, 128, 4, 128], F16, kind="ExternalInput")
    wv_t = nc.dram_tensor("wv", [CC, 128, 512], F16, kind="ExternalInput")
    wp_t = nc.dram_tensor("wp", [4, 128, 256], F16, kind="ExternalInput")
    bqk_t = nc.dram_tensor("bqk", [4, 128], F32, kind="ExternalInput")
    bv_t = nc.dram_tensor("bv", [512], F32, kind="ExternalInput")
    bp_t = nc.dram_tensor("bp", [256], F32, kind="ExternalInput")
    e_t = nc.dram_tensor("etab", [H, MC, 128, N], F16, kind="ExternalInput")
    out_t = nc.dram_tensor("out", [NB, N, DIM], F32, kind="ExternalOutput")

    aps = [t.ap() for t in (x_t, wqk_t, wv_t, wp_t, bqk_t, bv_t, bp_t, e_t, out_t)]
    with tile.TileContext(nc) as tc:
        _emit(tc, aps)
    nc.compile()
    return nc


def prep_inputs(inputs):
    """Host-side prep: fold norm affine + scale into weights, pack q/k rows
    into 32-row strips for PE row-tiling, and materialize E = exp(bias)."""
    x = np.asarray(inputs["x"], np.float32)
    norm_w = np.asarray(inputs["norm_w"], np.float32)
    norm_b = np.asarray(inputs["norm_b"], np.float32)
    qkv_w = np.asarray(inputs["qkv_w"], np.float32)
    qkv_b = np.asarray(inputs["qkv_b"], np.float32)
    proj_w = np.asarray(inputs["proj_w"], np.float32)
    proj_b = np.asarray(inputs["proj_b"], np.float32)
    ab = np.asarray(inputs["attn_biases"], np.float32)
    bi = np.asarray(inputs["bias_idxs"], np.int64)

    scale = KD ** -0.5
    wr = qkv_w.reshape(H, 2 * KD + D, DIM)
    br = qkv_b.reshape(H, 2 * KD + D)
    # fold norm_w into weights, norm_b into biases
    w_eff = wr * norm_w[None, None, :]
    b_eff = br + wr @ norm_b
    w_q = w_eff[:, :KD] * scale
    b_q = b_eff[:, :KD] * scale
    w_k = w_eff[:, KD:2 * KD]
    b_k = b_eff[:, KD:2 * KD]
    w_v = w_eff[:, 2 * KD:]
    b_v = b_eff[:, 2 * KD:]

    wqk = np.zeros((CC, 128, 4, 128), np.float16)
    bqk = np.zeros((4, 128), np.float32)
    for jt in range(4):
        kind_q = jt % 2 == 1
        hg = jt // 2
        w_src = w_q if kind_q else w_k
        b_src = b_q if kind_q else b_k
        for hp in range(4):
            h = hg * 4 + hp
            w_jc = w_src[h]  # [KD, DIM]
            for cc in range(CC):
                wqk[cc, :, jt, 32 * hp:32 * hp + KD] = (
                    w_jc[:, cc * 128:(cc + 1) * 128].T.astype(np.float16)
                )
            bqk[jt, 32 * hp:32 * hp + KD] = b_src[h]

    wv = np.zeros((CC, 128, 512), np.float16)
    for cc in range(CC):
        # [512(h,d), 128] -> [128, 512]
        wv[cc] = w_v.reshape(512, DIM)[:, cc * 128:(cc + 1) * 128].T.astype(np.float16)
    bv = b_v.reshape(512).astype(np.float32)

    wp = np.zeros((4, 128, 256), np.float16)
    for cc2 in range(4):
        wp[cc2] = proj_w[:, cc2 * 128:(cc2 + 1) * 128].T.astype(np.float16)
    bp = proj_b.astype(np.float32)

    # additive relative-position bias B.T (symmetric), folded into the score
    # PSUM on-chip via identity-weight matmuls before the exp
    etab = ab[:, bi].astype(np.float16).reshape(H, MC, 128, N)

    shared = {
        "wqk": wqk, "wv": wv, "wp": wp,
        "bqk": bqk, "bv": bv, "bp": bp, "etab": etab,
    }
    in_maps = []
    for c in range(NCORES):
        m = dict(shared)
        m["x"] = np.ascontiguousarray(x[c * NB:(c + 1) * NB])
        in_maps.append(m)
    return in_maps


_NC_CACHE = None


def _get_nc():
    global _NC_CACHE
    if _NC_CACHE is None:
        _NC_CACHE = build_module()
    return _NC_CACHE


def run(inputs, **spmd_kwargs):
    nc = _get_nc()
    in_maps = prep_inputs(inputs)
    res = bass_utils.run_bass_kernel_spmd(
        nc, in_maps, core_ids=list(range(NCORES)), **spmd_kwargs
    )
    out = np.concatenate([res.results[c]["out"] for c in range(NCORES)], axis=0)
    return out.astype(np.float32), res


def kernel(**inputs):
    out, _ = run(inputs)
    return out


if __name__ == "__main__":
    print("building module...")
    nc = _get_nc()
    print("instructions:", sum(len(f.basicblocks[0].instructions)
                               for f in nc.m.functions if f.basicblocks))


# revision 28
# speedup vs baseline: 1.1980x; 1.1980x over previous
"""Fused sparse-attention kernel for TRN2, SPMD over 8 NeuronCores.

Sharding: data-parallel over batch (32 -> 4 per core). Per core, the full
block (LayerNorm -> fused qkv -> per-head attention with gathered relative
position bias -> proj) is computed on-chip; attention probabilities never
touch HBM.

Softmax: softmax(S + B) is computed as exp(S + B - c); the gathered bias
B.T (precomputed on host from the tiny attn_biases table and the fixed
index map) is accumulated into the score PSUM by an identity-weight matmul
so the exp comes straight from PSUM with no extra element-wise pass;
row-sums are folded into the PV matmul via a ones-column appended to V.
The constant offset c gives fp16 headroom and cancels in the
normalization.

v2 schedule (vs baseline): the per-mc chain S -> exp -> mult -> PV is
software-pipelined at head (hp) granularity so the ACT engine (exp, the
bottleneck) runs back-to-back and the PE never sits behind a stalled PV:
S(mc+1) is emitted BEFORE PV(mc) on the tensor queue. Score PSUM tiles are
per-hp so exp(mc,hp0) frees banks while exp(mc,hp1) still runs. The softmax
reciprocals are batched ([2,N] per b, reciprocal_approx_fast) instead of 64
slow [1,N] reciprocal() calls, and PSUM drains moved off the ACT engine.
"""

import os
import sys

import numpy as np

for _p in ("/opt/trn_rl_repo", "/root/.axon_site/_ro/trn_rl_repo"):
    if os.path.isdir(_p) and _p not in sys.path:
        sys.path.insert(0, _p)

import concourse.bacc as bacc
import concourse.tile as tile
from concourse import bass_utils, mybir
from concourse.masks import make_identity

F32 = mybir.dt.float32
F16 = mybir.dt.float16

NCORES = 8
B_TOTAL = 32
NB = B_TOTAL // NCORES  # local batch per core
N = 1024
NT = 8        # 128-row tiles over n
DIM = 256
CC = 2        # 128-row chunks over DIM
H = 8
KD = 16
D = 64
MC = 8        # 128-row chunks over m
EPS = 1e-5
OFF = float(4.0 * np.log(2.0))  # exp offset for fp16 headroom (cancels)

MULT = mybir.AluOpType.mult
ADD = mybir.AluOpType.add


def _emit(tc, aps):
    nc = tc.nc
    x, wqk, wv, wp, bqk, bv, bp, etab, out = aps

    with tc.tile_pool(name="persist", bufs=1) as persist:
        # --- constants / weights resident in SBUF ---
        wqk_sb = persist.tile([128, CC, 4, 128], F16)
        nc.sync.dma_start(out=wqk_sb, in_=wqk.rearrange("cc ci jt j -> ci cc jt j"))
        wv_sb = persist.tile([128, CC, 512], F16)
        nc.sync.dma_start(out=wv_sb, in_=wv.rearrange("cc ci v -> ci cc v"))
        wp_sb = persist.tile([128, 4, 256], F16)
        nc.sync.dma_start(out=wp_sb, in_=wp.rearrange("cc ci c -> ci cc c"))
        bqk_sb = persist.tile([128, 4], F32)
        nc.sync.dma_start(out=bqk_sb, in_=bqk.rearrange("jt j -> j jt"))
        bv_sb = persist.tile([128, 512], F32)
        nc.sync.dma_start(out=bv_sb, in_=bv.partition_broadcast(128))
        bp_sb = persist.tile([128, 256], F32)
        nc.sync.dma_start(out=bp_sb, in_=bp.partition_broadcast(128))
        ident = persist.tile([128, 128], F16)
        make_identity(nc, ident)
        negoff = persist.tile([128, 1], F32)
        nc.vector.memset(negoff, -OFF)
        epsv = persist.tile([128, 1], F32)
        nc.vector.memset(epsv, EPS)

        qkT_l = []  # per-b [128, 4, 1024] f16: jt tiles (kT g0, qT g0, kT g1, qT g1)
        v_l = []    # per-b [128, NT, H, 65] f16: V rows + ones column per head
        ot_l = []   # per-b [128, 4, 1024] f16: O.T (dh on partitions, 4 chunks)

        # ---------------- phase 1: LN, xn.T, qkv projections ----------------
        with (
            tc.tile_pool(name="p1", bufs=2) as p1,
            tc.tile_pool(name="p1ps", bufs=2, space="PSUM") as p1ps,
        ):
            for b in range(NB):
                x_sb = p1.tile([128, NT, DIM], F32, tag="x", bufs=4)
                nc.sync.dma_start(
                    out=x_sb, in_=x[b].rearrange("(t p) c -> p t c", p=128)
                )
                xn_sb = p1.tile([128, NT, DIM], F16, tag="xn", bufs=2)
                for t in range(NT):
                    stats = p1.tile([128, 6], F32, tag="stats", bufs=3)
                    nc.vector.bn_stats(out=stats, in_=x_sb[:, t])
                    mv = p1.tile([128, 2], F32, tag="mv", bufs=3)
                    nc.vector.bn_aggr(out=mv, in_=stats)
                    rstd = p1.tile([128, 1], F32, tag="rstd", bufs=3)
                    nc.scalar.activation(
                        out=rstd, in_=mv[:, 1:2],
                        func=mybir.ActivationFunctionType.Sqrt,
                        bias=epsv, scale=1.0,
                    )
                    nc.vector.reciprocal(out=rstd, in_=rstd)
                    nc.vector.tensor_scalar(
                        out=xn_sb[:, t], in0=x_sb[:, t],
                        scalar1=mv[:, 0:1], scalar2=rstd,
                        op0=mybir.AluOpType.subtract, op1=mybir.AluOpType.mult,
                    )
                # xn.T via PE transpose; drain on ACT (phase 1 is DVE-bound,
                # ACT idles here)
                xnT = p1.tile([128, CC, N], F16, tag="xnt", bufs=2)
                for cc in range(CC):
                    for t in range(NT):
                        tp = p1ps.tile([128, 128], F16, tag="tp", bufs=2)
                        nc.tensor.transpose(
                            tp, xn_sb[:, t, cc * 128:(cc + 1) * 128], ident
                        )
                        nc.scalar.copy(
                            out=xnT[:, cc, t * 128:(t + 1) * 128], in_=tp
                        )
                # q.T / k.T, packed by 32-row strips per head (zeros padding)
                qkT = persist.tile([128, 4, N], F16, tag="qkT", bufs=NB, name="qkT")
                for jt in range(4):
                    qkp = p1ps.tile([128, N], F32, tag="qkp", bufs=2)
                    for nh in range(2):
                        for cc in range(CC):
                            nc.tensor.matmul(
                                qkp[:, nh * 512:(nh + 1) * 512],
                                lhsT=wqk_sb[:, cc, jt],
                                rhs=xnT[:, cc, nh * 512:(nh + 1) * 512],
                                start=(cc == 0), stop=(cc == CC - 1),
                            )
                    nc.scalar.activation(
                        out=qkT[:, jt], in_=qkp,
                        func=mybir.ActivationFunctionType.Identity,
                        bias=bqk_sb[:, jt:jt + 1], scale=1.0,
                    )
                qkT_l.append(qkT)
                # V (natural layout) + ones column, interleaved per head
                v_sb = persist.tile([128, NT, H, 65], F16, tag="v", bufs=NB,
                                    name="v_sb")
                nc.vector.memset(v_sb[:, :, :, 64:65], 1.0)
                for t in range(NT):
                    vp = p1ps.tile([128, 512], F32, tag="vp", bufs=2)
                    for cc in range(CC):
                        nc.tensor.matmul(
                            vp,
                            lhsT=xnT[:, cc, t * 128:(t + 1) * 128],
                            rhs=wv_sb[:, cc],
                            start=(cc == 0), stop=(cc == CC - 1),
                        )
                    nc.vector.tensor_tensor(
                        out=v_sb[:, t, :, 0:64],
                        in0=vp.rearrange("p (h d) -> p h d", d=64),
                        in1=bv_sb.rearrange("p (h d) -> p h d", d=64),
                        op=ADD,
                    )
                v_l.append(v_sb)

        # ---------------- phase 2: attention per head pair ----------------
        for b in range(NB):
            ot_l.append(persist.tile([128, 4, N], F16, tag="ot", bufs=NB,
                                     name="ot"))

        with (
            tc.tile_pool(name="p2", bufs=2) as p2,
            tc.tile_pool(name="p2ps", bufs=2, space="PSUM") as p2ps,
        ):
            for g in range(4):  # head pair {2g, 2g+1}
                # additive bias tiles B.T for this pair, quarter-granular so
                # the next g's DMA overlaps this g's tail
                e_tiles = {}
                for hp in range(2):
                    for q in range(2):
                        et = p2.tile([128, 4, N], F16, tag="e", bufs=4,
                                     name="et")
                        nc.sync.dma_start(
                            out=et,
                            in_=etab[2 * g + hp, 4 * q:4 * q + 4].rearrange(
                                "mc p n -> p mc n"),
                        )
                        e_tiles[(hp, q)] = et

                for b in range(NB):
                    # O'.T accumulators, one per head of the pair:
                    # [65, n] = V'.T @ P.T; row 64 carries the softmax sums
                    o_ts = [
                        p2ps.tile([65, N], F32, tag="o", bufs=2, name="o_ts")
                        for _ in range(2)
                    ]
                    s_tiles = {}

                    def emit_s(mc, b=b, s_tiles=s_tiles):
                        # scores S.T then the gathered bias B.T accumulated
                        # on top via identity-weight matmuls: exp reads
                        # S+B straight from PSUM, no DVE pass needed
                        for hp in range(2):
                            h = 2 * g + hp
                            jt = 2 * (h // 4)
                            strip = 32 * (h % 4)
                            s = p2ps.tile([128, N], F32, tag="s", bufs=2,
                                          name="s_ps")
                            for nh in range(2):
                                nc.tensor.matmul(
                                    s[:, nh * 512:(nh + 1) * 512],
                                    lhsT=qkT_l[b][strip:strip + KD, jt,
                                                  mc * 128:(mc + 1) * 128],
                                    rhs=qkT_l[b][strip:strip + KD, jt + 1,
                                                 nh * 512:(nh + 1) * 512],
                                    start=True, stop=False,
                                    skip_group_check=True,
                                    tile_position=(strip, 0),
                                )
                            for nh in range(2):
                                nc.tensor.matmul(
                                    s[:, nh * 512:(nh + 1) * 512],
                                    lhsT=ident,
                                    rhs=e_tiles[(hp, mc // 4)][
                                        :, mc % 4, nh * 512:(nh + 1) * 512],
                                    start=False, stop=True,
                                    skip_group_check=True,
                                )
                            s_tiles[(mc, hp)] = s

                    emit_s(0)
                    for mc in range(MC):
                        # keep the PE queue fed: next mc's scores go ahead of
                        # this mc's PV (which waits on exp)
                        if mc + 1 < MC:
                            emit_s(mc + 1)
                        ps_hp = []
                        for hp in range(2):
                            ps = p2.tile([128, N], F16, tag="ps", bufs=4,
                                         name="ps")
                            nc.scalar.activation(
                                out=ps, in_=s_tiles.pop((mc, hp)),
                                func=mybir.ActivationFunctionType.Exp,
                                bias=negoff, scale=1.0,
                            )
                            ps_hp.append(ps)
                        for hp in range(2):
                            for nh in range(2):
                                nc.tensor.matmul(
                                    o_ts[hp][:, nh * 512:(nh + 1) * 512],
                                    lhsT=v_l[b][:, mc, 2 * g + hp],
                                    rhs=ps_hp[hp][:, nh * 512:(nh + 1) * 512],
                                    start=(mc == 0), stop=(mc == MC - 1),
                                    skip_group_check=True,
                                )
                    # Drain PSUM fast on DVE, then batched-normalize:
                    # one reciprocal_approx_fast([2,N]) replaces two slow
                    # f16 reciprocal([1,N]) calls.
                    # softmax sums land on partitions 0 and 32 of a shared
                    # collector (compute APs need 32-aligned partition bases)
                    # so ONE reciprocal_approx_fast covers both heads; rows
                    # 1..31 hold garbage and are never read. The broadcast
                    # source must sit at partition base 0, so each head's
                    # reciprocal row is copied into its own [1, N] tile.
                    raws = []
                    c32 = p2.tile([33, N], F32, tag="c32", bufs=2)
                    for hp in range(2):
                        raw = p2.tile([64, N], F16, tag="raw", bufs=4)
                        # split the two PSUM drains across ACT and DVE so
                        # the o_ts banks free earlier at each b-transition
                        if hp == 0:
                            nc.scalar.copy(out=raw, in_=o_ts[hp][0:64])
                        else:
                            nc.vector.tensor_copy(out=raw, in_=o_ts[hp][0:64])
                        nc.vector.tensor_copy(
                            out=c32[32 * hp:32 * hp + 1], in_=o_ts[hp][64:65]
                        )
                        raws.append(raw)
                    r32 = p2.tile([33, N], F32, tag="r32", bufs=2)
                    nc.vector.reciprocal_approx_fast(out=r32, in_=c32)
                    for hp in range(2):
                        r1 = p2.tile([1, N], F16, tag="r1", bufs=4)
                        nc.vector.tensor_copy(
                            out=r1, in_=r32[32 * hp:32 * hp + 1])
                        rb = p2.tile([64, N], F16, tag="rb", bufs=4)
                        nc.gpsimd.partition_broadcast(rb, r1)
                        nc.vector.tensor_tensor(
                            out=ot_l[b][64 * hp:64 * hp + 64, g, :],
                            in0=raws[hp],
                            in1=rb,
                            op=MULT,
                        )

        # ---------------- phase 3: output projection ----------------
        with (
            tc.tile_pool(name="p3", bufs=2) as p3,
            tc.tile_pool(name="p3ps", bufs=4, space="PSUM") as p3ps,
        ):
            for b in range(NB):
                o_sb = p3.tile([128, NT, 256], F32, tag="osb", bufs=2)
                for nt in range(NT):
                    y = p3ps.tile([128, 256], F32, tag="y", bufs=4)
                    for cc2 in range(4):
                        nc.tensor.matmul(
                            y,
                            lhsT=ot_l[b][:, cc2, nt * 128:(nt + 1) * 128],
                            rhs=wp_sb[:, cc2],
                            start=(cc2 == 0), stop=(cc2 == 3),
                        )
                    nc.vector.tensor_tensor(
                        out=o_sb[:, nt], in0=y, in1=bp_sb, op=ADD
                    )
                nc.sync.dma_start(
                    out=out[b].rearrange("(t p) c -> p t c", p=128), in_=o_sb
                )


def build_module():
    nc = bacc.Bacc(
        "TRN2",
        target_bir_lowering=False,
        debug=False,
        enable_asserts=False,
        num_devices=NCORES,
    )
    x_t = nc.dram_tensor("x", [NB, N, DIM], F32, kind="ExternalInput")
    wqk_t = nc.dram_tensor("wqk", [CC, 128, 4, 128], F16, kind="ExternalInput")
    wv_t = nc.dram_tensor("wv", [CC, 128, 512], F16, kind="ExternalInput")
    wp_t = nc.dram_tensor("wp", [4, 128, 256], F16, kind="ExternalInput")
    bqk_t = nc.dram_tensor("bqk", [4, 128], F32, kind="ExternalInput")
    bv_t = nc.dram_tensor("bv", [512], F32, kind="ExternalInput")
    bp_t = nc.dram_tensor("bp", [256], F32, kind="ExternalInput")
    e_t = nc.dram_tensor("etab", [H, MC, 128, N], F16, kind="ExternalInput")
    out_t = nc.dram_tensor("out", [NB, N, DIM], F32, kind="ExternalOutput")

    aps = [t.ap() for t in (x_t, wqk_t, wv_t, wp_t, bqk_t, bv_t, bp_t, e_t, out_t)]
    with tile.TileContext(nc) as tc:
        _emit(tc, aps)
    nc.compile()
    return nc


def prep_inputs(inputs):
    """Host-side prep: fold norm affine + scale into weights, pack q/k rows
    into 32-row strips for PE row-tiling, and materialize E = exp(bias)."""
    x = np.asarray(inputs["x"], np.float32)
    norm_w = np.asarray(inputs["norm_w"], np.float32)
    norm_b = np.asarray(inputs["norm_b"], np.float32)
    qkv_w = np.asarray(inputs["qkv_w"], np.float32)
    qkv_b = np.asarray(inputs["qkv_b"], np.float32)
    proj_w = np.asarray(inputs["proj_w"], np.float32)
    proj_b = np.asarray(inputs["proj_b"], np.float32)
    ab = np.asarray(inputs["attn_biases"], np.float32)
    bi = np.asarray(inputs["bias_idxs"], np.int64)

    scale = KD ** -0.5
    wr = qkv_w.reshape(H, 2 * KD + D, DIM)
    br = qkv_b.reshape(H, 2 * KD + D)
    # fold norm_w into weights, norm_b into biases
    w_eff = wr * norm_w[None, None, :]
    b_eff = br + wr @ norm_b
    w_q = w_eff[:, :KD] * scale
    b_q = b_eff[:, :KD] * scale
    w_k = w_eff[:, KD:2 * KD]
    b_k = b_eff[:, KD:2 * KD]
    w_v = w_eff[:, 2 * KD:]
    b_v = b_eff[:, 2 * KD:]

    wqk = np.zeros((CC, 128, 4, 128), np.float16)
    bqk = np.zeros((4, 128), np.float32)
    for jt in range(4):
        kind_q = jt % 2 == 1
        hg = jt // 2
        w_src = w_q if kind_q else w_k
        b_src = b_q if kind_q else b_k
        for hp in range(4):
            h = hg * 4 + hp
            w_jc = w_src[h]  # [KD, DIM]
            for cc in range(CC):
                wqk[cc, :, jt, 32 * hp:32 * hp + KD] = (
                    w_jc[:, cc * 128:(cc + 1) * 128].T.astype(np.float16)
                )
            bqk[jt, 32 * hp:32 * hp + KD] = b_src[h]

    wv = np.zeros((CC, 128, 512), np.float16)
    for cc in range(CC):
        # [512(h,d), 128] -> [128, 512]
        wv[cc] = w_v.reshape(512, DIM)[:, cc * 128:(cc + 1) * 128].T.astype(np.float16)
    bv = b_v.reshape(512).astype(np.float32)

    wp = np.zeros((4, 128, 256), np.float16)
    for cc2 in range(4):
        wp[cc2] = proj_w[:, cc2 * 128:(cc2 + 1) * 128].T.astype(np.float16)
    bp = proj_b.astype(np.float32)

    # additive relative-position bias B.T (symmetric), folded into the score
    # PSUM on-chip via identity-weight matmuls before the exp
    etab = ab[:, bi].astype(np.float16).reshape(H, MC, 128, N)

    shared = {
        "wqk": wqk, "wv": wv, "wp": wp,
        "bqk": bqk, "bv": bv, "bp": bp, "etab": etab,
    }
    in_maps = []
    for c in range(NCORES):
        m = dict(shared)
        m["x"] = np.ascontiguousarray(x[c * NB:(c + 1) * NB])
        in_maps.append(m)
    return in_maps


_NC_CACHE = None


def _get_nc():
    global _NC_CACHE
    if _NC_CACHE is None:
        _NC_CACHE = build_module()
    return _NC_CACHE


def run(inputs, **spmd_kwargs):
    nc = _get_nc()
    in_maps = prep_inputs(inputs)
    res = bass_utils.run_bass_kernel_spmd(
        nc, in_maps, core_ids=list(range(NCORES)), **spmd_kwargs
    )
    out = np.concatenate([res.results[c]["out"] for c in range(NCORES)], axis=0)
    return out.astype(np.float32), res


def kernel(**inputs):
    out, _ = run(inputs)
    return out


if __name__ == "__main__":
    print("building module...")
    nc = _get_nc()
    print("instructions:", sum(len(f.basicblocks[0].instructions)
                               for f in nc.m.functions if f.basicblocks))
